# revision 19
# baseline (speedup 1.0000x reference)
"""NT-Xent loss, V4.1: symmetric-recompute block-circulant sharding.

Each core owns row-panels {c, c+8} (8 msubs of 128 rows).  Every msub computes
a 13-block column band q in [0,12] of the similarity matrix (fp8 DoubleRow
matmuls) and row-sums exp() of the whole band (free via accum).  Coverage of
q in [13,15] comes from the COLUMN sums of the q in [1,3] blocks, exported via
one wide GPSIMD partition_all_reduce per msub.

exp() is split across two independent engine chains, each with its own
double-buffered PSUM pool (PSUM fits 4096 fp32/partition total):
  - ScalarE (ACT) table-exp, 1536-wide slots (exact), with fused row-sum accum
  - custom DVE op EXP8SUM_ANT, 512-wide slots: [(x*C0+B)^2+G]^8 with a fused
    row-sum accumulator; the constant a2^8 factor is applied on the host.
The export slice (band cols [512,2048), q 1..3) is always the first ACT chunk
so one partition_all_reduce per msub covers it.  Pair positives + log +
combine run on host (f64).

Inputs are column-ROTATED per core so all programs are identical (SPMD-safe).
"""

import re
import operator

import numpy as np
import ml_dtypes

import concourse.bacc as bacc
import concourse.bass as bass
import concourse.mybir as mybir
import concourse.tile as tile
from concourse import bass_isa
from concourse.bass_utils import run_bass_kernel_spmd

B = 4096
TWO_B = 2 * B
D = 512
T = 0.5
NCORES = 8
PANEL = 512
FP8_SCALE = 16.0
EXP_SCALE = 1.0 / (FP8_SCALE * FP8_SCALE * T)   # 1/128
SELF_SIM = float(np.exp(1.0 / T))
F8 = mybir.dt.float8e4
F32 = mybir.dt.float32
BF16 = mybir.dt.bfloat16
NP_F8 = ml_dtypes.float8_e4m3
NP_BF16 = ml_dtypes.bfloat16

# ---- custom DVE exp constants: exp(x') ~ A8 * [ (x'*C0T + BETA)^2 + GAMMA ]^8
C0T = EXP_SCALE / 8.0
BETA = 1.015769459
GAMMA = 0.979217646
A8 = 3.745006884e-03

# ---- geometry -------------------------------------------------------------
# Per-msub band = 13 q-blocks (6656 rotated cols).  Band offset: 0 for A-msubs
# (gm 0-3, panel c), 4096 for B-msubs (gm 4-7, panel c+8); global zt column =
# (band_off + local) % 8192.  Export slice = band cols [512, 2048).
QBAND = 15 * 512            # 7680
EXPORT_LOCAL = 512
EXPORT_W = 512
# Per-msub chunks: each chunk = (blocks, engine, slot) where blocks is a tuple
# of band block indices (0..14; local cols [512b, 512b+512)).  ACT chunks take
# 3 blocks (1536-wide psum slot), DVE chunks 1 block (512 slot).  The export
# chunk is always blocks (1,2,3); only block 1's column sums are exported
# (hardware partition_all_reduce costs ~4 ns/col, so exports are minimized by
# computing 15 of 16 q-blocks per row).  Extra ACT triples are placed so every
# zt DMA arrival group is roughly ACT/DVE time-balanced.
# Global balance: 22 ACT x 1536 + 54 DVE x 512 = 61440 elem-cols per core.
EXTRA_TRIPLES = {
    0: [(4, 5, 6), (12, 13, 14)], 1: [(4, 5, 6), (12, 13, 14)],
    2: [(4, 5, 6), (12, 13, 14)], 3: [(4, 5, 6)],
    4: [(12, 13, 14), (4, 5, 6)], 5: [(12, 13, 14), (4, 5, 6)],
    6: [(12, 13, 14), (4, 5, 6)], 7: [(4, 5, 6)],
}

# rows outputs are packed: per-gm slot counts -> prefix offsets
# ACT counts (3,3,3,2,3,3,3,2); DVE counts (6,6,6,9,6,6,6,9)
ACT_SLOT_OFF = (0, 3, 6, 9, 11, 14, 17, 20)
N_ROWS_A_TOTAL = 22
# zt DMA pieces, in issue order: msub weights first (A then B panels), then
# the two export slices, then the rest.
ZT_PIECES = ((0, 512), (4096, 512), (512, 1536), (4608, 1536),
             (2048, 2048), (6144, 2048))
DVE_SLOT_OFF = (0, 6, 12, 18, 27, 33, 39, 45)
N_ROWS_D_TOTAL = 54


def _msub_chunks(gm):
    chunks = [((1, 2, 3), "act", 0)]                 # export chunk
    used = {1, 2, 3}
    for j, tri in enumerate(EXTRA_TRIPLES[gm]):
        chunks.append((tri, "act", 1 + j))
        used |= set(tri)
    slot = 0
    for b in range(15):
        if b not in used:
            chunks.append(((b,), "dve", slot))
            slot += 1
    return chunks


_EXP8_OP = None


def _register_exp8():
    """Register the fused DVE op out=[(x*C0+C1)^2+C2]^8, accum_out=rowsum."""
    global _EXP8_OP
    if _EXP8_OP is not None:
        return _EXP8_OP
    from concourse import dve_ops as _DO
    from concourse.dve_spec import Spec, Src0, C0, C1, C2
    from concourse.dve_table_gen import dve_ver_for

    if "EXP8SUM_ANT" in _DO.CUSTOM_DVE_SPECS:
        _EXP8_OP = next(op for op in _DO.OPS if op.name == "EXP8SUM_ANT")
        return _EXP8_OP

    _t = Src0 * C0
    _u = _t + C1
    _v = _u * _u
    _w = _v + C2
    _w2 = _w * _w
    _w4 = _w2 * _w2
    _w8 = _w4 * _w4

    def _ref(in0, in1, c0, c1, c2):
        x = np.asarray(in0, np.float32)
        c0 = np.float32(c0) if np.isscalar(c0) else np.asarray(c0, np.float32)
        c1 = np.float32(c1) if np.isscalar(c1) else np.asarray(c1, np.float32)
        t = (x * c0).astype(np.float32)
        u = (t + c1).astype(np.float32)
        v = (u * u).astype(np.float32)
        w = (v + np.float32(c2)).astype(np.float32)
        w2 = (w * w).astype(np.float32)
        w4 = (w2 * w2).astype(np.float32)
        w8 = (w4 * w4).astype(np.float32)
        return w8, w8.sum(axis=-1, keepdims=True).astype(np.float32)

    spec = Spec(body=_w8, accum=operator.add, reference=_ref)
    op = _DO.DveOp("EXP8SUM_ANT", spec, subdim=False, uops_sha={})
    _DO.OPS.append(op)
    _DO.CUSTOM_DVE_SPECS[op.name] = spec
    _DO._SUB_OPCODE_FOR_NAME[op.name] = max(_DO._SUB_OPCODE_FOR_NAME.values()) + 1
    ver = dve_ver_for("TRN2")
    try:
        op.compile(ver)
    except ValueError as e:
        m = re.search(r"([0-9a-f]{16})", str(e))
        op.uops_sha[ver] = m.group(1)
        op.compile(ver)
    _EXP8_OP = op
    return op


def _schedule():
    """All (gm, chunk) work items grouped by zt DMA-piece arrival; inside each
    group, greedily interleave ACT/DVE chunks by accumulated engine time so
    both exp chains stay fed and finish together."""
    # rank of each global 512-block under ZT_PIECES DMA order
    rank = {}
    for r, (n0, nw) in enumerate(ZT_PIECES):
        for g in range(n0, n0 + nw, 512):
            rank[g] = r

    def arrival(gm, c):
        off = 0 if gm < 4 else 4096
        return max(rank[(off + 512 * b) % TWO_B] for b in c[0])

    groups = {}
    for gm in range(8):
        for c in _msub_chunks(gm):
            groups.setdefault(arrival(gm, c), []).append((gm, c))
    items = []
    # ACT-first bias at the start: its chain is the critical path, and the
    # first zt slice DMA'd is the A-export data it needs.
    t_a, t_d = -0.01, 0.0
    for a in sorted(groups):
        acts = sorted((it for it in groups[a] if it[1][1] == "act"),
                      key=lambda it: (it[1][0], it[0]))
        dves = sorted((it for it in groups[a] if it[1][1] == "dve"),
                      key=lambda it: (it[1][0], it[0]))
        while acts or dves:
            if dves and (not acts or t_d <= t_a):
                gm, c = dves.pop(0)
                t_d += 0.66
            else:
                gm, c = acts.pop(0)
                t_a += 1.65
            items.append((a, gm, c))
    return items


def _build_nc(repeats: int = 1, loop: int = 0):
    """repeats>1 unrolls the body N times (timing variant).  loop=R instead
    wraps one body in a hardware For_i executing R times - arbitrarily large
    R without instruction-memory growth."""
    exp8 = _register_exp8()
    nc = bacc.Bacc("TRN2", target_bir_lowering=False, debug=False)

    zt_d = nc.dram_tensor("zt8", [2, 128, 2, TWO_B], F8, kind="ExternalInput")
    rowsA_d = nc.dram_tensor("rowsA", [128, N_ROWS_A_TOTAL], F32, kind="ExternalOutput")
    rowsD_d = nc.dram_tensor("rowsD", [128, N_ROWS_D_TOTAL], F32, kind="ExternalOutput")
    cols_d = nc.dram_tensor("cols", [8, EXPORT_W], F32, kind="ExternalOutput")

    sched = _schedule()

    with tile.TileContext(nc) as tc:
        with (
            tc.tile_pool(name="zn", bufs=1) as zn_pool,
            tc.tile_pool(name="small", bufs=1) as small_pool,
            tc.tile_pool(name="psA", bufs=2, space=bass.MemorySpace.PSUM) as psA_pool,
            tc.tile_pool(name="psD", bufs=2, space=bass.MemorySpace.PSUM) as psD_pool,
            tc.tile_pool(name="sc0", bufs=3) as sc0_pool,     # export sc (p_a_r in)
            tc.tile_pool(name="scA", bufs=2) as scA_pool,     # other ACT outs
            tc.tile_pool(name="scD", bufs=3) as scD_pool,     # DVE outs
            tc.tile_pool(name="csr", bufs=2) as csr_pool,     # p_a_r results
        ):
            zt = zn_pool.tile([128, 2, 2, TWO_B], F8)
            rows_a = small_pool.tile([128, N_ROWS_A_TOTAL], F32)
            rows_d = small_pool.tile([128, N_ROWS_D_TOTAL], F32)

            import contextlib
            loop_cm = tc.For_i(0, loop) if loop else contextlib.nullcontext()
            with loop_cm:
              for _rep in range(repeats):
                # inputs on both HWDGE queues (SP + ACT); weight columns and
                # export slices first so both exp chains start early
                for n0, nw in ZT_PIECES:
                    nc.sync.dma_start(
                        zt[:, 0, :, n0:n0 + nw], zt_d[0][:, :, n0:n0 + nw])
                    nc.scalar.dma_start(
                        zt[:, 1, :, n0:n0 + nw], zt_d[1][:, :, n0:n0 + nw])

                for _arr, gm, (blocks, eng, slot) in sched:
                    off = 0 if gm < 4 else 4096
                    w = 512 * len(blocks)
                    if eng == "act":
                        ps = psA_pool.tile([128, 1536], F32, tag="psA")
                    else:
                        ps = psD_pool.tile([128, 512], F32, tag="psD")
                    wcol = gm * 128 if gm < 4 else 4096 + (gm - 4) * 128
                    for c in range(2):
                        wgt = zt[:, c, :, wcol:wcol + 128]
                        for nb, b in enumerate(blocks):
                            n0 = (off + 512 * b) % TWO_B
                            nc.tensor.matmul(
                                ps[:, nb * 512:(nb + 1) * 512],
                                wgt,
                                zt[:, c, :, n0:n0 + 512],
                                start=(c == 0),
                                stop=(c == 1),
                                perf_mode=mybir.MatmulPerfMode.DoubleRow,
                            )
                    if eng == "act":
                        is_export = slot == 0
                        pool = sc0_pool if is_export else scA_pool
                        sc = pool.tile([128, 1536], BF16,
                                       tag="sc0" if is_export else "scA")
                        nc.scalar.activation(
                            sc[:, :w], ps[:, :w],
                            mybir.ActivationFunctionType.Exp,
                            scale=EXP_SCALE,
                            accum_out=rows_a[:, ACT_SLOT_OFF[gm] + slot:
                                             ACT_SLOT_OFF[gm] + slot + 1],
                        )
                        if is_export:
                            csr = csr_pool.tile([128, EXPORT_W], F32, tag="csr")
                            nc.gpsimd.partition_all_reduce(
                                csr[:], sc[:, :EXPORT_W],
                                channels=128,
                                reduce_op=bass_isa.ReduceOp.add,
                            )
                            nc.sync.dma_start(cols_d[gm], csr[:1])
                    else:
                        sc = scD_pool.tile([128, 512], BF16, tag="scD")
                        nc.vector._custom_dve(
                            exp8,
                            out=sc[:, :w],
                            in0=ps[:, :w],
                            s0=C0T, s1=BETA, imm2=GAMMA,
                            accum_out=rows_d[:, DVE_SLOT_OFF[gm] + slot:
                                             DVE_SLOT_OFF[gm] + slot + 1],
                        )

                nc.sync.dma_start(rowsA_d.ap(), rows_a[:])
                nc.sync.dma_start(rowsD_d.ap(), rows_d[:])

    nc.compile()
    return nc


_CACHE = {}


def _get_nc():
    if "nc" not in _CACHE:
        _CACHE["nc"] = _build_nc()
    return _CACHE["nc"]


def _quantize(z_i, z_j):
    z = np.concatenate([np.asarray(z_i), np.asarray(z_j)], axis=0).astype(np.float32)
    norms = np.sqrt((z * z).sum(axis=1, dtype=np.float32))
    zn = z / np.maximum(norms, 1e-8)[:, None]
    zq = (zn * FP8_SCALE).astype(NP_F8)      # [8192, 512] fp8 rows
    return zn, zq


def _dr_layout(zq_cols):
    """[rows(=512 dims), cols] fp8 -> DoubleRow [c, p, j, col]; d=256c+128j+p."""
    return np.ascontiguousarray(
        zq_cols.reshape(2, 2, 128, zq_cols.shape[1]).transpose(0, 2, 1, 3))


def make_inputs(z_i, z_j):
    zn, zq = _quantize(z_i, z_j)
    zqT = zq.T                                 # [512, 8192], d-major
    in_maps = []
    for core in range(NCORES):
        rot = np.roll(zqT, -PANEL * core, axis=1)    # rotated cols
        in_maps.append({"zt8": _dr_layout(rot)})
    return in_maps, zn


def finish(results, zn) -> np.ndarray:
    denom = np.zeros(TWO_B, dtype=np.float64)
    for core, res in enumerate(results):
        rows_a = res["rowsA"].astype(np.float64)
        rows_d = res["rowsD"].astype(np.float64)
        cols = res["cols"].astype(np.float64)
        for gm in range(8):
            s = np.zeros(128, dtype=np.float64)
            for blocks, eng, slot in _msub_chunks(gm):
                if eng == "act":
                    s += rows_a[:, ACT_SLOT_OFF[gm] + slot]
                else:
                    s += rows_d[:, DVE_SLOT_OFF[gm] + slot] * A8
            base = (PANEL * core if gm < 4 else B + PANEL * core) + 128 * (gm % 4)
            denom[base:base + 128] += s
            # export = band cols [512, 2048) of this msub (always ACT, no A8)
            roff0 = 0 if gm < 4 else B
            g0 = (PANEL * core + roff0 + EXPORT_LOCAL) % TWO_B
            v = cols[gm]
            end = g0 + EXPORT_W
            if end <= TWO_B:
                denom[g0:end] += v
            else:
                k = TWO_B - g0
                denom[g0:] += v[:k]
                denom[:end - TWO_B] += v[k:]
    denom -= SELF_SIM
    logs = np.log(denom + 1e-8)
    pair_sum = float(np.einsum("ij,ij->", zn[:B].astype(np.float64),
                               zn[B:].astype(np.float64)))
    loss = (logs.sum() - (2.0 / T) * pair_sum) / TWO_B
    return np.array(loss, dtype=np.float32)


def kernel(z_i: np.ndarray, z_j: np.ndarray) -> np.ndarray:
    nc = _get_nc()
    in_maps, zn = make_inputs(z_i, z_j)
    res = run_bass_kernel_spmd(nc, in_maps, list(range(NCORES)))
    return finish(res.results, zn)


# ---------- numpy model of one core's outputs (for CoreSim checks) ----------

def _exp8_np(x):
    x = np.asarray(x, np.float32)
    t = (x * np.float32(C0T)).astype(np.float32)
    u = (t + np.float32(BETA)).astype(np.float32)
    v = (u * u).astype(np.float32)
    w = (v + np.float32(GAMMA)).astype(np.float32)
    w2 = (w * w).astype(np.float32)
    w4 = (w2 * w2).astype(np.float32)
    return (w4 * w4).astype(np.float32)


def expected_core_outputs(in_maps, core):
    m = in_maps[core]
    zt = m["zt8"].astype(np.float32).transpose(0, 2, 1, 3).reshape(512, TWO_B)
    rows_a = np.zeros((128, N_ROWS_A_TOTAL), dtype=np.float32)
    rows_d = np.zeros((128, N_ROWS_D_TOTAL), dtype=np.float32)
    cols = np.zeros((8, EXPORT_W), dtype=np.float32)
    for gm in range(8):
        off = 0 if gm < 4 else 4096
        wcol = gm * 128 if gm < 4 else 4096 + (gm - 4) * 128
        w8 = zt[:, wcol:wcol + 128]
        for blocks, eng, slot in _msub_chunks(gm):
            cols_idx = [(off + 512 * b + i) % TWO_B
                        for b in blocks for i in range(512)]
            dots = w8.T @ zt[:, cols_idx]
            if eng == "act":
                e = np.exp(dots.astype(np.float32) * np.float32(EXP_SCALE))
                rows_a[:, ACT_SLOT_OFF[gm] + slot] = e.sum(1, dtype=np.float32)
                if slot == 0:
                    eb = e[:, :EXPORT_W].astype(NP_BF16).astype(np.float32)
                    cols[gm] = eb.sum(0, dtype=np.float32)
            else:
                e = _exp8_np(dots)
                rows_d[:, DVE_SLOT_OFF[gm] + slot] = e.sum(1, dtype=np.float32)
    return {"rowsA": rows_a, "rowsD": rows_d, "cols": cols}


if __name__ == "__main__":
    rng = np.random.default_rng(0)
    z_i = rng.standard_normal((B, D), dtype=np.float32)
    z_j = rng.standard_normal((B, D), dtype=np.float32)
    in_maps, zn = make_inputs(z_i, z_j)
    fake = [expected_core_outputs(in_maps, c) for c in range(NCORES)]
    loss_model = finish(fake, zn)
    z = np.concatenate([z_i, z_j], 0).astype(np.float64)
    n = np.linalg.norm(z, axis=-1)
    sim = (z @ z.T) / np.maximum(n[:, None] * n[None, :], 1e-8) / T
    pos = np.concatenate([np.diagonal(sim, B), np.diagonal(sim, -B)])
    dn = ((1.0 - np.eye(TWO_B)) * np.exp(sim)).sum(1)
    ref = np.mean(np.log(dn + 1e-8) - pos)
    print(f"model={loss_model:.7f} ref={ref:.7f} rel={abs(loss_model-ref)/abs(ref):.3e}")


# revision 37
# speedup vs baseline: 1.6745x; 1.6745x over previous
"""NT-Xent loss, V4.4: full-row sharding, dual-engine exp, no collective ops.

Each core owns row-panels {c, c+8} (8 msubs of 128 rows).  Every msub computes
ALL 16 column blocks of its similarity rows (fp8 DoubleRow matmuls) and
row-sums exp() of the full row via fused accumulators -- denominators complete
per-row on the owning core; no column-sum exports (HW partition_all_reduce
measured ~5us/op makes any Pool-based symmetric reuse slower than simply
recomputing mirror blocks).

exp() is split across two independent engine chains, each with its own
double-buffered PSUM pool (PSUM fits 4096 fp32/partition total):
  - ScalarE (ACT) table-exp, 1536-wide slots (exact; measured ~0.65 ns/elem
    with bf16 out), with fused row-sum accum -- 28 chunks/core
  - custom DVE op EXP8SUM_ANT, 512-wide slots: [(x*C0+B)^2+G]^8 with a fused
    row-sum accumulator (measured 658 ns/op); a2^8 applied on host -- 44/core
Stationary weights are columns of the rotated zt8 already in SBUF (no separate
weights input).  Inputs stream over both HWDGE queues (SP + ACT), chunk
emission ordered by DMA arrival and greedily time-balanced across chains.
Pair positives + log + combine run on host (f64).

Inputs are column-ROTATED per core so all programs are identical (SPMD-safe).
"""

import re
import operator

import numpy as np
import ml_dtypes

import concourse.bacc as bacc
import concourse.bass as bass
import concourse.mybir as mybir
import concourse.tile as tile
from concourse import bass_isa
from concourse.bass_utils import run_bass_kernel_spmd

B = 4096
TWO_B = 2 * B
D = 512
T = 0.5
NCORES = 8
PANEL = 512
FP8_SCALE = 16.0
EXP_SCALE = 1.0 / (FP8_SCALE * FP8_SCALE * T)   # 1/128
SELF_SIM = float(np.exp(1.0 / T))
F8 = mybir.dt.float8e4
F32 = mybir.dt.float32
BF16 = mybir.dt.bfloat16
NP_F8 = ml_dtypes.float8_e4m3
NP_BF16 = ml_dtypes.bfloat16

# ---- custom DVE exp constants: exp(x') ~ A8 * [ (x'*C0T + BETA)^2 + GAMMA ]^8
C0T = EXP_SCALE / 8.0
BETA = 1.015769459
GAMMA = 0.979217646
A8 = 3.745006884e-03

# ---- geometry -------------------------------------------------------------
# Per-msub band = 13 q-blocks (6656 rotated cols).  Band offset: 0 for A-msubs
# (gm 0-3, panel c), 4096 for B-msubs (gm 4-7, panel c+8); global zt column =
# (band_off + local) % 8192.  Export slice = band cols [512, 2048).
QBAND = 16 * 512            # 8192: full rows -- no colsum exports, no Pool
EXPORT_LOCAL = 512
EXPORT_W = 512
NO_EXPORT = True
# Per-msub chunks: each chunk = (blocks, engine, slot) where blocks is a tuple
# of band block indices (0..14; local cols [512b, 512b+512)).  ACT chunks take
# 3 blocks (1536-wide psum slot), DVE chunks 1 block (512 slot).  The export
# chunk is always blocks (1,2,3); only block 1's column sums are exported
# (hardware partition_all_reduce costs ~4 ns/col, so exports are minimized by
# computing 15 of 16 q-blocks per row).  Extra ACT triples are placed so every
# zt DMA arrival group is roughly ACT/DVE time-balanced.
# Global balance: 27 ACT x 1536 + 39 DVE x 512 = 61440 elem-cols per core
# (measured HW rates: ACT ~0.65 ns/elem with bf16 out, custom DVE ~1.29).
EXTRA_TRIPLES = {
    0: [(4, 5, 6), (7, 8, 9), (12, 13, 14)], 1: [(4, 5, 6), (12, 13, 14)],
    2: [(4, 5, 6), (7, 8, 9), (12, 13, 14)], 3: [(4, 5, 6), (12, 13, 14)],
    4: [(12, 13, 14), (4, 5, 6), (7, 8, 9)], 5: [(12, 13, 14), (4, 5, 6)],
    6: [(12, 13, 14), (4, 5, 6), (7, 8, 9)], 7: [(4, 5, 6), (12, 13, 14)],
}

# rows outputs are packed: per-gm slot counts -> prefix offsets
# ACT counts (4,3,4,3,4,3,4,3); DVE counts (4,7,4,7,4,7,4,7)
ACT_SLOT_OFF = (0, 4, 7, 11, 14, 18, 21, 25)
N_ROWS_A_TOTAL = 28
# zt DMA pieces, in issue order: msub weights first (A then B panels), then
# the two export slices, then the rest.
ZT_PIECES = ((0, 512), (4096, 512), (512, 1536), (4608, 1536),
             (2048, 2048), (6144, 2048))
DVE_SLOT_OFF = (0, 4, 11, 15, 22, 26, 33, 37)
N_ROWS_D_TOTAL = 44


_CHUNKS_OVERRIDE = None     # timing probes: dict gm -> chunk list


def _msub_chunks(gm):
    if _CHUNKS_OVERRIDE is not None:
        return _CHUNKS_OVERRIDE[gm]
    chunks = [((1, 2, 3), "act", 0)]                 # export chunk
    used = {1, 2, 3}
    for j, tri in enumerate(EXTRA_TRIPLES[gm]):
        chunks.append((tri, "act", 1 + j))
        used |= set(tri)
    slot = 0
    for b in range(16):
        if b not in used:
            chunks.append(((b,), "dve", slot))
            slot += 1
    return chunks


_EXP8_OP = None


def _register_exp8():
    """Register the fused DVE op out=[(x*C0+C1)^2+C2]^8, accum_out=rowsum."""
    global _EXP8_OP
    if _EXP8_OP is not None:
        return _EXP8_OP
    from concourse import dve_ops as _DO
    from concourse.dve_spec import Spec, Src0, C0, C1, C2
    from concourse.dve_table_gen import dve_ver_for

    if "EXP8SUM_ANT" in _DO.CUSTOM_DVE_SPECS:
        _EXP8_OP = next(op for op in _DO.OPS if op.name == "EXP8SUM_ANT")
        return _EXP8_OP

    _t = Src0 * C0
    _u = _t + C1
    _v = _u * _u
    _w = _v + C2
    _w2 = _w * _w
    _w4 = _w2 * _w2
    _w8 = _w4 * _w4

    def _ref(in0, in1, c0, c1, c2):
        x = np.asarray(in0, np.float32)
        c0 = np.float32(c0) if np.isscalar(c0) else np.asarray(c0, np.float32)
        c1 = np.float32(c1) if np.isscalar(c1) else np.asarray(c1, np.float32)
        t = (x * c0).astype(np.float32)
        u = (t + c1).astype(np.float32)
        v = (u * u).astype(np.float32)
        w = (v + np.float32(c2)).astype(np.float32)
        w2 = (w * w).astype(np.float32)
        w4 = (w2 * w2).astype(np.float32)
        w8 = (w4 * w4).astype(np.float32)
        return w8, w8.sum(axis=-1, keepdims=True).astype(np.float32)

    spec = Spec(body=_w8, accum=operator.add, reference=_ref)
    op = _DO.DveOp("EXP8SUM_ANT", spec, subdim=False, uops_sha={})
    _DO.OPS.append(op)
    _DO.CUSTOM_DVE_SPECS[op.name] = spec
    _DO._SUB_OPCODE_FOR_NAME[op.name] = max(_DO._SUB_OPCODE_FOR_NAME.values()) + 1
    ver = dve_ver_for("TRN2")
    try:
        op.compile(ver)
    except ValueError as e:
        m = re.search(r"([0-9a-f]{16})", str(e))
        op.uops_sha[ver] = m.group(1)
        op.compile(ver)
    _EXP8_OP = op
    return op


def _schedule():
    """All (gm, chunk) work items grouped by zt DMA-piece arrival; inside each
    group, greedily interleave ACT/DVE chunks by accumulated engine time so
    both exp chains stay fed and finish together."""
    # rank of each global 512-block under ZT_PIECES DMA order
    rank = {}
    for r, (n0, nw) in enumerate(ZT_PIECES):
        for g in range(n0, n0 + nw, 512):
            rank[g] = r

    def arrival(gm, c):
        off = 0 if gm < 4 else 4096
        return max(rank[(off + 512 * b) % TWO_B] for b in c[0])

    groups = {}
    for gm in range(8):
        for c in _msub_chunks(gm):
            groups.setdefault(arrival(gm, c), []).append((gm, c))
    items = []
    # ACT-first bias at the start: its chain is the critical path, and the
    # first zt slice DMA'd is the A-export data it needs.
    t_a, t_d = -0.01, 0.0
    for a in sorted(groups):
        acts = sorted((it for it in groups[a] if it[1][1] == "act"),
                      key=lambda it: (it[1][0], it[0]))
        dves = sorted((it for it in groups[a] if it[1][1] == "dve"),
                      key=lambda it: (it[1][0], it[0]))
        while acts or dves:
            if dves and (not acts or t_d <= t_a):
                gm, c = dves.pop(0)
                t_d += 0.66
            else:
                gm, c = acts.pop(0)
                t_a += 1.0
            items.append((a, gm, c))
    return items


def _build_nc(repeats: int = 1, loop: int = 0, zt_bufs: int = 1):
    """repeats>1 unrolls the body N times (timing variant).  loop=R instead
    wraps one body in a hardware For_i executing R times - arbitrarily large
    R without instruction-memory growth.  zt_bufs=2 double-buffers the input
    tile across repeats (timing-only: isolates cross-rep DMA serialization)."""
    exp8 = _register_exp8()
    nc = bacc.Bacc("TRN2", target_bir_lowering=False, debug=False)

    zt_d = nc.dram_tensor("zt8", [2, 128, 2, TWO_B], F8, kind="ExternalInput")
    n_rows_a = [sum(1 for c in _msub_chunks(g) if c[1] == "act") for g in range(8)]
    n_rows_d = [sum(1 for c in _msub_chunks(g) if c[1] == "dve") for g in range(8)]
    act_off = [sum(n_rows_a[:g]) for g in range(8)]
    dve_off = [sum(n_rows_d[:g]) for g in range(8)]
    rowsA_d = nc.dram_tensor("rowsA", [128, max(1, sum(n_rows_a))], F32, kind="ExternalOutput")
    rowsD_d = nc.dram_tensor("rowsD", [128, max(1, sum(n_rows_d))], F32, kind="ExternalOutput")
    cols_d = None if NO_EXPORT else nc.dram_tensor(
        "cols", [8, EXPORT_W], F32, kind="ExternalOutput")

    sched = _schedule()

    with tile.TileContext(nc) as tc:
        with (
            tc.tile_pool(name="zn", bufs=zt_bufs) as zn_pool,
            tc.tile_pool(name="small", bufs=1) as small_pool,
            tc.tile_pool(name="psA", bufs=2, space=bass.MemorySpace.PSUM) as psA_pool,
            tc.tile_pool(name="psD", bufs=2, space=bass.MemorySpace.PSUM) as psD_pool,
            tc.tile_pool(name="sc0", bufs=3) as sc0_pool,     # export sc (p_a_r in)
            tc.tile_pool(name="scA", bufs=2) as scA_pool,     # other ACT outs
            tc.tile_pool(name="scD", bufs=3) as scD_pool,     # DVE outs
            tc.tile_pool(name="csr", bufs=2) as csr_pool,     # p_a_r results
        ):
            rows_a = small_pool.tile([128, max(1, sum(n_rows_a))], F32)
            rows_d = small_pool.tile([128, max(1, sum(n_rows_d))], F32)

            import contextlib
            loop_cm = tc.For_i(0, loop) if loop else contextlib.nullcontext()
            with loop_cm:
              for _rep in range(repeats):
                zt = zn_pool.tile([128, 2, 2, TWO_B], F8)
                # inputs on both HWDGE queues (SP + ACT); weight columns and
                # export slices first so both exp chains start early
                for n0, nw in ZT_PIECES:
                    nc.sync.dma_start(
                        zt[:, 0, :, n0:n0 + nw], zt_d[0][:, :, n0:n0 + nw])
                    nc.scalar.dma_start(
                        zt[:, 1, :, n0:n0 + nw], zt_d[1][:, :, n0:n0 + nw])

                for _arr, gm, (blocks, eng, slot) in sched:
                    off = 0 if gm < 4 else 4096
                    w = 512 * len(blocks)
                    if eng == "act":
                        ps = psA_pool.tile([128, 1536], F32, tag="psA")
                    else:
                        ps = psD_pool.tile([128, 512], F32, tag="psD")
                    wcol = gm * 128 if gm < 4 else 4096 + (gm - 4) * 128
                    for c in range(2):
                        wgt = zt[:, c, :, wcol:wcol + 128]
                        for nb, b in enumerate(blocks):
                            n0 = (off + 512 * b) % TWO_B
                            nc.tensor.matmul(
                                ps[:, nb * 512:(nb + 1) * 512],
                                wgt,
                                zt[:, c, :, n0:n0 + 512],
                                start=(c == 0),
                                stop=(c == 1),
                                perf_mode=mybir.MatmulPerfMode.DoubleRow,
                            )
                    if eng == "act":
                        is_export = slot == 0
                        pool = sc0_pool if is_export else scA_pool
                        sc = pool.tile([128, 1536], BF16,
                                       tag="sc0" if is_export else "scA")
                        nc.scalar.activation(
                            sc[:, :w], ps[:, :w],
                            mybir.ActivationFunctionType.Exp,
                            scale=EXP_SCALE,
                            accum_out=rows_a[:, act_off[gm] + slot:
                                             act_off[gm] + slot + 1],
                        )
                        if is_export and not NO_EXPORT:
                            csr = csr_pool.tile([128, EXPORT_W], F32, tag="csr")
                            nc.gpsimd.partition_all_reduce(
                                csr[:], sc[:, :EXPORT_W],
                                channels=128,
                                reduce_op=bass_isa.ReduceOp.add,
                            )
                            nc.sync.dma_start(cols_d[gm], csr[:1])
                    else:
                        sc = scD_pool.tile([128, 512], BF16, tag="scD")
                        nc.vector._custom_dve(
                            exp8,
                            out=sc[:, :w],
                            in0=ps[:, :w],
                            s0=C0T, s1=BETA, imm2=GAMMA,
                            accum_out=rows_d[:, dve_off[gm] + slot:
                                             dve_off[gm] + slot + 1],
                        )

                if sum(n_rows_a):
                    nc.sync.dma_start(rowsA_d.ap(), rows_a[:])
                if sum(n_rows_d):
                    nc.sync.dma_start(rowsD_d.ap(), rows_d[:])

    _dedup_ldweights(nc)
    nc.compile()
    return nc


def _dedup_ldweights(nc):
    """Drop InstLdweights whose stationary weights are identical to the
    previous load on the in-order PE stream (bass emits one per matmul
    unconditionally).  Only sync-free loads are removed; any other PE
    instruction type resets the tracked state."""
    for bb in nc.m.functions[0].blocks:
        last_key = None
        kept = []
        for ins in bb.instructions:
            if isinstance(ins, mybir.InstLdweights):
                si = ins.sync_info
                has_sync = si is not None and (si.on_wait or si.on_update)
                key = str(ins.ins[0])
                if key == last_key and not has_sync:
                    continue
                last_key = key
            elif getattr(ins, "engine", None) == mybir.EngineType.PE and \
                    not isinstance(ins, (mybir.InstMatmult,
                                         mybir.InstEventSemaphore)):
                last_key = None
            kept.append(ins)
        if len(kept) != len(bb.instructions):
            bb.instructions = kept


_CACHE = {}


def _get_nc():
    if "nc" not in _CACHE:
        _CACHE["nc"] = _build_nc()
    return _CACHE["nc"]


def _quantize(z_i, z_j):
    z = np.concatenate([np.asarray(z_i), np.asarray(z_j)], axis=0).astype(np.float32)
    norms = np.sqrt((z * z).sum(axis=1, dtype=np.float32))
    zn = z / np.maximum(norms, 1e-8)[:, None]
    zq = (zn * FP8_SCALE).astype(NP_F8)      # [8192, 512] fp8 rows
    return zn, zq


def _dr_layout(zq_cols):
    """[rows(=512 dims), cols] fp8 -> DoubleRow [c, p, j, col]; d=256c+128j+p."""
    return np.ascontiguousarray(
        zq_cols.reshape(2, 2, 128, zq_cols.shape[1]).transpose(0, 2, 1, 3))


def make_inputs(z_i, z_j):
    zn, zq = _quantize(z_i, z_j)
    zqT = zq.T                                 # [512, 8192], d-major
    in_maps = []
    for core in range(NCORES):
        rot = np.roll(zqT, -PANEL * core, axis=1)    # rotated cols
        in_maps.append({"zt8": _dr_layout(rot)})
    return in_maps, zn


def finish(results, zn) -> np.ndarray:
    denom = np.zeros(TWO_B, dtype=np.float64)
    for core, res in enumerate(results):
        rows_a = res["rowsA"].astype(np.float64)
        rows_d = res["rowsD"].astype(np.float64)
        cols = None if NO_EXPORT else res["cols"].astype(np.float64)
        for gm in range(8):
            s = np.zeros(128, dtype=np.float64)
            for blocks, eng, slot in _msub_chunks(gm):
                if eng == "act":
                    s += rows_a[:, ACT_SLOT_OFF[gm] + slot]
                else:
                    s += rows_d[:, DVE_SLOT_OFF[gm] + slot] * A8
            base = (PANEL * core if gm < 4 else B + PANEL * core) + 128 * (gm % 4)
            denom[base:base + 128] += s
            if not NO_EXPORT:
                # export = block-1 colsums of this msub (always ACT, no A8)
                roff0 = 0 if gm < 4 else B
                g0 = (PANEL * core + roff0 + EXPORT_LOCAL) % TWO_B
                v = cols[gm]
                end = g0 + EXPORT_W
                if end <= TWO_B:
                    denom[g0:end] += v
                else:
                    k = TWO_B - g0
                    denom[g0:] += v[:k]
                    denom[:end - TWO_B] += v[k:]
    denom -= SELF_SIM
    logs = np.log(denom + 1e-8)
    pair_sum = float(np.einsum("ij,ij->", zn[:B].astype(np.float64),
                               zn[B:].astype(np.float64)))
    loss = (logs.sum() - (2.0 / T) * pair_sum) / TWO_B
    return np.array(loss, dtype=np.float32)


def kernel(z_i: np.ndarray, z_j: np.ndarray) -> np.ndarray:
    nc = _get_nc()
    in_maps, zn = make_inputs(z_i, z_j)
    res = run_bass_kernel_spmd(nc, in_maps, list(range(NCORES)))
    return finish(res.results, zn)


# ---------- numpy model of one core's outputs (for CoreSim checks) ----------

def _exp8_np(x):
    x = np.asarray(x, np.float32)
    t = (x * np.float32(C0T)).astype(np.float32)
    u = (t + np.float32(BETA)).astype(np.float32)
    v = (u * u).astype(np.float32)
    w = (v + np.float32(GAMMA)).astype(np.float32)
    w2 = (w * w).astype(np.float32)
    w4 = (w2 * w2).astype(np.float32)
    return (w4 * w4).astype(np.float32)


def expected_core_outputs(in_maps, core):
    m = in_maps[core]
    zt = m["zt8"].astype(np.float32).transpose(0, 2, 1, 3).reshape(512, TWO_B)
    rows_a = np.zeros((128, N_ROWS_A_TOTAL), dtype=np.float32)
    rows_d = np.zeros((128, N_ROWS_D_TOTAL), dtype=np.float32)
    cols = np.zeros((8, EXPORT_W), dtype=np.float32)   # unused when NO_EXPORT
    for gm in range(8):
        off = 0 if gm < 4 else 4096
        wcol = gm * 128 if gm < 4 else 4096 + (gm - 4) * 128
        w8 = zt[:, wcol:wcol + 128]
        for blocks, eng, slot in _msub_chunks(gm):
            cols_idx = [(off + 512 * b + i) % TWO_B
                        for b in blocks for i in range(512)]
            dots = w8.T @ zt[:, cols_idx]
            if eng == "act":
                e = np.exp(dots.astype(np.float32) * np.float32(EXP_SCALE))
                rows_a[:, ACT_SLOT_OFF[gm] + slot] = e.sum(1, dtype=np.float32)
                if slot == 0 and not NO_EXPORT:
                    eb = e[:, :EXPORT_W].astype(NP_BF16).astype(np.float32)
                    cols[gm] = eb.sum(0, dtype=np.float32)
            else:
                e = _exp8_np(dots)
                rows_d[:, DVE_SLOT_OFF[gm] + slot] = e.sum(1, dtype=np.float32)
    if NO_EXPORT:
        return {"rowsA": rows_a, "rowsD": rows_d}
    return {"rowsA": rows_a, "rowsD": rows_d, "cols": cols}


if __name__ == "__main__":
    rng = np.random.default_rng(0)
    z_i = rng.standard_normal((B, D), dtype=np.float32)
    z_j = rng.standard_normal((B, D), dtype=np.float32)
    in_maps, zn = make_inputs(z_i, z_j)
    fake = [expected_core_outputs(in_maps, c) for c in range(NCORES)]
    loss_model = finish(fake, zn)
    z = np.concatenate([z_i, z_j], 0).astype(np.float64)
    n = np.linalg.norm(z, axis=-1)
    sim = (z @ z.T) / np.maximum(n[:, None] * n[None, :], 1e-8) / T
    pos = np.concatenate([np.diagonal(sim, B), np.diagonal(sim, -B)])
    dn = ((1.0 - np.eye(TWO_B)) * np.exp(sim)).sum(1)
    ref = np.mean(np.log(dn + 1e-8) - pos)
    print(f"model={loss_model:.7f} ref={ref:.7f} rel={abs(loss_model-ref)/abs(ref):.3e}")
